# revision 1
# baseline (speedup 1.0000x reference)
"""Trainium2 Bass kernel for nn_AverageAttention: cumulative-average attention
with a sigmoid gating Linear(2D->2D).

Strategy: data-parallel over batch (B=8 = one batch element per NeuronCore).
All on-chip work happens in transposed space ([feature, token]) because the
TensorEngine contracts over the partition dim:
  - host passes x^T per core; cumsum along tokens = free-dim scan on VectorE
  - gates^T[o,t] = sum_k wT[k,o]^T @ G^T[k,t] with W stationary (host
    pre-transposes W and casts to bf16), G = concat(x, avg) resident in SBUF
    as bf16
  - sigmoid+bias fused on ScalarE reading PSUM, combine on VectorE,
    outputs written transposed and un-transposed on host.
"""
import sys

if "/opt/trn_rl_repo" not in sys.path:
    sys.path.insert(0, "/opt/trn_rl_repo")

import numpy as np
import ml_dtypes

B, T, D = 8, 2048, 2048
O = 2 * D          # gate output features (4096)
P = 128            # partitions
KT = D // P        # 16 k-tiles per half of G
DT = D // P        # 16 output-feature tiles
NK = 2 * KT        # 32 k-tiles total
TS = 512           # t-slice (matmul moving free dim)
NS = T // TS       # 4 t-slices

_compiled = None


def _build():
    import concourse.mybir as mybir
    import concourse.tile as tile
    from concourse import bacc

    f32 = mybir.dt.float32
    bf16 = mybir.dt.bfloat16

    nc = bacc.Bacc(trn_type="TRN2", target_bir_lowering=False, debug=False,
                   num_devices=B)

    xT_d = nc.declare_dram_parameter("xT", [D, T], f32, isOutput=False)
    wT_d = nc.declare_dram_parameter("wT", [O, O], bf16, isOutput=False)
    bias_d = nc.declare_dram_parameter("bias", [O], f32, isOutput=False)
    inv_d = nc.declare_dram_parameter("inv_t", [1, T], f32, isOutput=False)
    avgT_d = nc.declare_dram_parameter("avgT", [D, T], f32, isOutput=True)
    outT_d = nc.declare_dram_parameter("outT", [D, T], f32, isOutput=True)

    with tile.TileContext(nc) as tc:
        with tc.tile_pool(name="consts", bufs=1) as consts, \
             tc.tile_pool(name="xin", bufs=2) as xin, \
             tc.tile_pool(name="resid", bufs=1) as resid, \
             tc.tile_pool(name="wpool", bufs=2) as wpool, \
             tc.tile_pool(name="sigp", bufs=4) as sigp, \
             tc.tile_pool(name="outp", bufs=4) as outp, \
             tc.tile_pool(name="psum", bufs=8, space="PSUM") as pp:

            inv_sb = consts.tile([P, T], f32)
            nc.sync.dma_start(out=inv_sb, in_=inv_d[:, :].to_broadcast((P, T)))
            bias_sb = consts.tile([P, O // P], f32)
            nc.sync.dma_start(
                out=bias_sb, in_=bias_d.rearrange("(c p) -> p c", p=P))

            xT_bf = resid.tile([P, KT, T], bf16)
            avgT_bf = resid.tile([P, KT, T], bf16)

            # ---- Phase A: load x^T, cumavg scan, write avg^T, cast bf16 ----
            for j in range(KT):
                xf = xin.tile([P, T], f32, tag="xf")
                nc.sync.dma_start(out=xf, in_=xT_d[j * P:(j + 1) * P, :])
                nc.scalar.copy(xT_bf[:, j, :], xf)
                # in-place cumsum along t, then in-place * 1/(t+1)
                nc.vector.tensor_tensor_scan(
                    out=xf, data0=xf, data1=xf, initial=0.0,
                    op0=mybir.AluOpType.add, op1=mybir.AluOpType.bypass)
                nc.vector.tensor_mul(xf, xf, inv_sb)
                nc.sync.dma_start(out=avgT_d[j * P:(j + 1) * P, :], in_=xf)
                nc.scalar.copy(avgT_bf[:, j, :], xf)

            # ---- Phase B: gating matmul + sigmoid + combine ----
            wT_r = wT_d[:, :].rearrange("(kt p) o -> p kt o", p=P)
            for i in range(DT):
                w_i = wpool.tile([P, 2 * NK, P], bf16, tag="w")
                nc.sync.dma_start(
                    out=w_i[:, 0:NK, :],
                    in_=wT_r[:, :, i * P:(i + 1) * P])
                nc.sync.dma_start(
                    out=w_i[:, NK:2 * NK, :],
                    in_=wT_r[:, :, D + i * P:D + (i + 1) * P])
                for s in range(NS):
                    sl = slice(s * TS, (s + 1) * TS)
                    psum_ig = pp.tile([P, TS], f32, tag="ps")
                    psum_fg = pp.tile([P, TS], f32, tag="ps")
                    for k in range(NK):
                        rhs = (xT_bf[:, k, sl] if k < KT
                               else avgT_bf[:, k - KT, sl])
                        nc.tensor.matmul(psum_ig, lhsT=w_i[:, k, :], rhs=rhs,
                                         start=(k == 0), stop=(k == NK - 1))
                    for k in range(NK):
                        rhs = (xT_bf[:, k, sl] if k < KT
                               else avgT_bf[:, k - KT, sl])
                        nc.tensor.matmul(psum_fg, lhsT=w_i[:, NK + k, :],
                                         rhs=rhs,
                                         start=(k == 0), stop=(k == NK - 1))
                    sig_i = sigp.tile([P, TS], f32, tag="sig")
                    nc.scalar.activation(
                        sig_i, psum_ig, mybir.ActivationFunctionType.Sigmoid,
                        bias=bias_sb[:, i:i + 1])
                    sig_f = sigp.tile([P, TS], f32, tag="sig")
                    nc.scalar.activation(
                        sig_f, psum_fg, mybir.ActivationFunctionType.Sigmoid,
                        bias=bias_sb[:, KT + i:KT + i + 1])
                    out_s = outp.tile([P, TS], f32, tag="out")
                    nc.vector.tensor_mul(out_s, sig_i, xT_bf[:, i, sl])
                    nc.vector.tensor_mul(sig_f, sig_f, avgT_bf[:, i, sl])
                    nc.vector.tensor_add(out_s, out_s, sig_f)
                    nc.sync.dma_start(
                        out=outT_d[i * P:(i + 1) * P, sl], in_=out_s)

    nc.compile()
    return nc


def _get_compiled():
    global _compiled
    if _compiled is None:
        _compiled = _build()
    return _compiled


def _run(inputs, trace=False, **spmd_kwargs):
    from concourse.bass_utils import run_bass_kernel_spmd

    nc = _get_compiled()
    layer_in = np.asarray(inputs["layer_in"], dtype=np.float32)
    W_gate = np.asarray(inputs["W_gate"], dtype=np.float32)
    b_gate = np.asarray(inputs["b_gate"], dtype=np.float32)

    wT = np.ascontiguousarray(W_gate.T).astype(ml_dtypes.bfloat16)
    inv_t = (1.0 / np.arange(1, T + 1, dtype=np.float32)).reshape(1, T)

    in_maps = []
    for b in range(B):
        in_maps.append({
            "xT": np.ascontiguousarray(layer_in[b].T),
            "wT": wT,
            "bias": b_gate,
            "inv_t": inv_t,
        })

    res = run_bass_kernel_spmd(nc, in_maps, core_ids=list(range(B)),
                               trace=trace, **spmd_kwargs)
    gating = np.empty((B, T, D), dtype=np.float32)
    avg = np.empty((B, T, D), dtype=np.float32)
    for b in range(B):
        gating[b] = res.results[b]["outT"].T
        avg[b] = res.results[b]["avgT"].T
    return (gating, avg), res


def kernel(**inputs):
    (gating, avg), _ = _run(inputs, trace=False)
    return gating, avg


# revision 2
# speedup vs baseline: 1.0995x; 1.0995x over previous
"""Trainium2 Bass kernel for nn_AverageAttention: cumulative-average attention
with a sigmoid gating Linear(2D->2D).

Strategy: data-parallel over batch (B=8 = one batch element per NeuronCore).
All on-chip work happens in transposed space ([feature, token]) because the
TensorEngine contracts over the partition dim:
  - host passes x^T per core (bf16); cumsum along tokens = free-dim scans on
    VectorE, chunked into 512-column slices (chained via a carry tile) and
    ordered slice-first so the gating matmul can start consuming avg tiles
    almost immediately
  - gates^T[o,t] = sum_k wT[k,o]^T @ G^T[k,t] with W stationary (host
    pre-transposes W and casts to bf16), G = concat(x, avg) resident in SBUF
    as bf16
  - sigmoid+bias fused on ScalarE reading PSUM, combine on VectorE,
    outputs written transposed and un-transposed on host.
"""
import sys

if "/opt/trn_rl_repo" not in sys.path:
    sys.path.insert(0, "/opt/trn_rl_repo")

import numpy as np
import ml_dtypes

B, T, D = 8, 2048, 2048
O = 2 * D          # gate output features (4096)
P = 128            # partitions
KT = D // P        # 16 k-tiles per half of G
DT = D // P        # 16 output-feature tiles
NK = 2 * KT        # 32 k-tiles total
TS = 512           # t-slice (matmul moving free dim / scan chunk)
NS = T // TS       # 4 t-slices

_compiled = None


def _build():
    import concourse.mybir as mybir
    import concourse.tile as tile
    from concourse import bacc

    f32 = mybir.dt.float32
    bf16 = mybir.dt.bfloat16

    nc = bacc.Bacc(trn_type="TRN2", target_bir_lowering=False, debug=False,
                   num_devices=B)

    xT_d = nc.declare_dram_parameter("xT", [D, T], bf16, isOutput=False)
    wT_d = nc.declare_dram_parameter("wT", [O, O], bf16, isOutput=False)
    bias_d = nc.declare_dram_parameter("bias", [O], f32, isOutput=False)
    inv_d = nc.declare_dram_parameter("inv_t", [1, T], f32, isOutput=False)
    avgT_d = nc.declare_dram_parameter("avgT", [D, T], f32, isOutput=True)
    outT_d = nc.declare_dram_parameter("outT", [D, T], f32, isOutput=True)

    with tile.TileContext(nc) as tc:
        with tc.tile_pool(name="consts", bufs=1) as consts, \
             tc.tile_pool(name="resid", bufs=1) as resid, \
             tc.tile_pool(name="csp", bufs=3) as csp, \
             tc.tile_pool(name="avp", bufs=3) as avp, \
             tc.tile_pool(name="wpool", bufs=2) as wpool, \
             tc.tile_pool(name="sigp", bufs=4) as sigp, \
             tc.tile_pool(name="outp", bufs=4) as outp, \
             tc.tile_pool(name="psum", bufs=8, space="PSUM") as pp:

            inv_sb = consts.tile([P, T], f32)
            nc.sync.dma_start(out=inv_sb, in_=inv_d[:, :].to_broadcast((P, T)))
            bias_sb = consts.tile([P, O // P], f32)
            nc.sync.dma_start(
                out=bias_sb, in_=bias_d.rearrange("(c p) -> p c", p=P))
            carry = consts.tile([P, KT], f32)

            xT_bf = resid.tile([P, KT, T], bf16)
            avgT_bf = resid.tile([P, KT, T], bf16)

            for j in range(KT):
                nc.sync.dma_start(out=xT_bf[:, j, :],
                                  in_=xT_d[j * P:(j + 1) * P, :])

            # ---- Phase A: chunked cumavg scans, slice-major order ----
            for s in range(NS):
                sl = slice(s * TS, (s + 1) * TS)
                for j in range(KT):
                    xc = xT_bf[:, j, sl]
                    cs = csp.tile([P, TS], f32, tag="cs")
                    nc.vector.tensor_tensor_scan(
                        out=cs, data0=xc, data1=xc,
                        initial=(0.0 if s == 0 else carry[:, j:j + 1]),
                        op0=mybir.AluOpType.add, op1=mybir.AluOpType.bypass)
                    if s < NS - 1:
                        nc.vector.tensor_copy(carry[:, j:j + 1],
                                              cs[:, TS - 1:TS])
                    av = avp.tile([P, TS], f32, tag="av")
                    nc.vector.tensor_mul(av, cs, inv_sb[:, sl])
                    nc.sync.dma_start(out=avgT_d[j * P:(j + 1) * P, sl],
                                      in_=av)
                    nc.scalar.copy(avgT_bf[:, j, sl], av)

            # ---- Phase B: gating matmul + sigmoid + combine ----
            wT_r = wT_d[:, :].rearrange("(kt p) o -> p kt o", p=P)
            for i in range(DT):
                w_i = wpool.tile([P, 2 * NK, P], bf16, tag="w")
                nc.sync.dma_start(
                    out=w_i[:, 0:NK, :],
                    in_=wT_r[:, :, i * P:(i + 1) * P])
                nc.sync.dma_start(
                    out=w_i[:, NK:2 * NK, :],
                    in_=wT_r[:, :, D + i * P:D + (i + 1) * P])
                for s in range(NS):
                    sl = slice(s * TS, (s + 1) * TS)
                    psum_ig = pp.tile([P, TS], f32, tag="ps")
                    psum_fg = pp.tile([P, TS], f32, tag="ps")
                    for k in range(NK):
                        rhs = (xT_bf[:, k, sl] if k < KT
                               else avgT_bf[:, k - KT, sl])
                        nc.tensor.matmul(psum_ig, lhsT=w_i[:, k, :], rhs=rhs,
                                         start=(k == 0), stop=(k == NK - 1))
                    for k in range(NK):
                        rhs = (xT_bf[:, k, sl] if k < KT
                               else avgT_bf[:, k - KT, sl])
                        nc.tensor.matmul(psum_fg, lhsT=w_i[:, NK + k, :],
                                         rhs=rhs,
                                         start=(k == 0), stop=(k == NK - 1))
                    sig_i = sigp.tile([P, TS], f32, tag="sig")
                    nc.scalar.activation(
                        sig_i, psum_ig, mybir.ActivationFunctionType.Sigmoid,
                        bias=bias_sb[:, i:i + 1])
                    sig_f = sigp.tile([P, TS], f32, tag="sig")
                    nc.scalar.activation(
                        sig_f, psum_fg, mybir.ActivationFunctionType.Sigmoid,
                        bias=bias_sb[:, KT + i:KT + i + 1])
                    out_s = outp.tile([P, TS], f32, tag="out")
                    nc.vector.tensor_mul(out_s, sig_i, xT_bf[:, i, sl])
                    nc.vector.tensor_mul(sig_f, sig_f, avgT_bf[:, i, sl])
                    nc.vector.tensor_add(out_s, out_s, sig_f)
                    nc.sync.dma_start(
                        out=outT_d[i * P:(i + 1) * P, sl], in_=out_s)

    nc.compile()
    return nc


def _get_compiled():
    global _compiled
    if _compiled is None:
        _compiled = _build()
    return _compiled


def _run(inputs, trace=False, **spmd_kwargs):
    from concourse.bass_utils import run_bass_kernel_spmd

    nc = _get_compiled()
    layer_in = np.asarray(inputs["layer_in"], dtype=np.float32)
    W_gate = np.asarray(inputs["W_gate"], dtype=np.float32)
    b_gate = np.asarray(inputs["b_gate"], dtype=np.float32)

    wT = np.ascontiguousarray(W_gate.T).astype(ml_dtypes.bfloat16)
    inv_t = (1.0 / np.arange(1, T + 1, dtype=np.float32)).reshape(1, T)

    in_maps = []
    for b in range(B):
        in_maps.append({
            "xT": np.ascontiguousarray(layer_in[b].T).astype(ml_dtypes.bfloat16),
            "wT": wT,
            "bias": b_gate,
            "inv_t": inv_t,
        })

    res = run_bass_kernel_spmd(nc, in_maps, core_ids=list(range(B)),
                               trace=trace, **spmd_kwargs)
    gating = np.empty((B, T, D), dtype=np.float32)
    avg = np.empty((B, T, D), dtype=np.float32)
    for b in range(B):
        gating[b] = res.results[b]["outT"].T
        avg[b] = res.results[b]["avgT"].T
    return (gating, avg), res


def kernel(**inputs):
    (gating, avg), _ = _run(inputs, trace=False)
    return gating, avg
